# revision 35
# baseline (speedup 1.0000x reference)
"""CRF partition-function kernel for Trainium2 (8 NeuronCores).

Probe/rank-1 splice algorithm:
  logZ = lse(alpha_{T-1}) with alpha_t = D_t E^T alpha_{t-1},
  E = exp(trans - c0), D_t = diag(exp(emit_t - m_t)) (host-centered).
  A product of >=~10 of these positive transfer matrices is numerically
  rank-1 (Perron collapse), so each chunk product P_c (L=8 factors) is
  fully described by two probe vectors:
      v_c = P_c @ 1   (forward vector scan)
      r_c = P_c^T @ 1 (backward vector scan)
  with P_c ~= v_c r_c^T / (1^T v_c), and
      logZ = log(r_1^T alpha_host) + sum_c log(r_{c+1}^T v_c / 1^T v_c)
             + sum over device factors (m_f + c0) + host-chunk part.
  This replaces the T*NT^3 matrix scan with 2*T*NT^2 of batched
  matvecs.  T=8192 -> 2048 chunks of L=4; chunk 0 (3 factors, exact
  BOS start) runs on the host in f64; the other 2047 forward and 2047
  backward chains run as 8 cores x 512 lockstep chains x 4 rounds
  (cores 0-3 forward, 4-7 backward; identical program, different
  inputs -- the backward recurrence z <- E(d*z) is re-shaped to
  MM-then-scale by a one-column emission shift, and each chain's init
  vector rides in the demi tensor as a leading block).

Round (per chain group g of 256 chains): one PSUM tile [128, 512]
accumulates 4 matmuls (2 j-tiles x 2 k-halves), then one VectorE
tensor_mul applies the per-(state,chain) emission column and writes
the bf16 state.  A dummy-matmul warmup burst opens the PE HAM clock
gate during the input DMAs; demi streams on 3 DMA queues in
need-order; qout is dumped per group across 3 queues.

Dtypes: weights fp8e4 (rescaled so max ~200), emissions fp8e4 (the
2^-wk compensation is left out so the bf16 state grows 2^wk/round;
host constants account for it), state bf16, PSUM f32.  Validated vs
f64 reference: ~-10 nats on logZ ~53616 (tolerance 2e-2 rel ~= 1070).
"""

import numpy as np
import ml_dtypes

import concourse.bass as bass
import concourse.bacc as bacc
import concourse.mybir as mybir
import concourse.tile as tile
from concourse.bass_utils import run_bass_kernel_spmd

BF16 = ml_dtypes.bfloat16
FP8 = ml_dtypes.float8_e4m3

NT = 256
T_FULL = 8192
N_CORES = 8
P = 128
L = 4             # rounds (chunk length)
C = T_FULL // L   # 2048 chunks (chunk 0 on host)
NCH = 512         # chain slots per core
G = 2             # chain groups (latency hiding)
GCH = NCH // G
RB = 2            # demi round-blocks per group
RL = L // RB
W_DT = "fp8"      # "fp8" | "bf16"

_CACHE = {}


def build_nc(nonce=""):
    f32 = mybir.dt.float32
    bf16 = mybir.dt.bfloat16
    wdt = mybir.dt.float8e4 if W_DT == "fp8" else bf16
    GW = 2 * GCH          # state/psum width per group
    DBW = RL * GW         # demi block width

    fp8 = mybir.dt.float8e4
    nc = bacc.Bacc(None, target_bir_lowering=False)
    # w: col = (kh*2+jt)*128 + j'   (lhsT blocks)
    w = nc.declare_dram_parameter("w" + nonce, [P, 4 * P], wdt, isOutput=False)
    # demi: per group [init | round 0 | ... | round L-1] blocks of GW cols,
    # each block col = h*GCH + ch.  Init doubles as the round-0 state.
    # fp8, with the 2^-wk weight compensation left out (the bf16 state
    # simply grows 2^wk per round; host constants account for it).
    demi = nc.declare_dram_parameter(
        "demi", [P, G * (L + 1) * GW], fp8, isOutput=False)
    qout = nc.declare_dram_parameter("qout", [P, G * GW], bf16, isOutput=True)

    with tile.TileContext(nc) as tc:
        with (
            tc.tile_pool(name="const", bufs=1) as cp,
            tc.tile_pool(name="state", bufs=1) as sp,
            tc.tile_pool(name="psA", bufs=2, space=bass.MemorySpace.PSUM) as ppA,
            tc.tile_pool(name="psB", bufs=2, space=bass.MemorySpace.PSUM) as ppB,
        ):
            # PE warmup burst: ~4us of dummy matmuls (no data deps) so the
            # HAM clock gate opens before the real rounds start
            wu = sp.tile([P, 5 * P], bf16, tag="wu", name="wu")
            nc.vector.memset(wu[:], 1.0)
            wups = ppA.tile([P, 4 * P], f32, tag="wups", name="wups")
            for _ in range(6):
                nc.tensor.matmul(wups[:], wu[:, 0:P], wu[:, P:5 * P],
                                 start=True, stop=True)

            Wt = cp.tile([P, 4 * P], wdt, tag="w", name="w")
            nc.sync.dma_start(Wt[:], w[:, :])

            S = [[sp.tile([P, GW], bf16, tag=f"s{g}{ph}", name=f"s{g}{ph}")
                  for ph in range(2)] for g in range(G)]
            OUT = sp.tile([P, G * GW], bf16, tag="out", name="out")

            # demi: issued in need-order per group (init gates round 0);
            # sync takes the last block so the two group queues stay short
            GB = (L + 1) * GW
            Dinit = [cp.tile([P, GW], fp8, tag=f"di{g}", name=f"di{g}")
                     for g in range(G)]
            D = [[cp.tile([P, DBW], fp8, tag=f"d{g}{b}", name=f"d{g}{b}")
                  for b in range(RB)] for g in range(G)]
            dq = {(0, 0): nc.scalar, (0, 1): nc.scalar,
                  (1, 0): nc.gpsimd, (1, 1): nc.sync}
            for g in range(G):
                eng = nc.scalar if g == 0 else nc.sync
                eng.dma_start(Dinit[g][:], demi[:, g * GB:g * GB + GW])
            for b in range(RB):
                for g in range(G):
                    dq[(g, b)].dma_start(
                        D[g][b][:],
                        demi[:, g * GB + (1 + b * RL) * GW:
                             g * GB + (1 + (b + 1) * RL) * GW])

            pools = [ppA, ppB]
            for i in range(L):
                b, ri = divmod(i, RL)
                PS = [pools[g].tile([P, GW], f32, tag=f"ps{g}", name=f"ps{g}")
                      for g in range(G)]
                for g in range(G):
                    for kh, jt in ((0, 0), (1, 0), (0, 1), (1, 1)):
                        wsl = Wt[:, (kh * 2 + jt) * P:(kh * 2 + jt + 1) * P]
                        if i == 0:
                            rhs = Dinit[g][:, kh * GCH:(kh + 1) * GCH]
                        else:
                            rhs = S[g][(i + 1) % 2][:, kh * GCH:(kh + 1) * GCH]
                        nc.tensor.matmul(
                            PS[g][:, jt * GCH:(jt + 1) * GCH],
                            wsl, rhs,
                            start=(kh == 0), stop=(kh == 1))
                for g in range(G):
                    dst = (OUT[:, g * GW:(g + 1) * GW] if i == L - 1
                           else S[g][i % 2][:])
                    nc.vector.tensor_mul(
                        dst, PS[g][:],
                        D[g][b][:, ri * GW:(ri + 1) * GW])
                    if i == L - 1:
                        # dump this group's half as soon as its TT lands;
                        # split g0 across two queues for parallel rings
                        o0 = g * GW
                        if g == 0:
                            h = GW // 2
                            nc.scalar.dma_start(qout[:, o0:o0 + h],
                                                OUT[:, o0:o0 + h])
                            nc.gpsimd.dma_start(qout[:, o0 + h:o0 + GW],
                                                OUT[:, o0 + h:o0 + GW])
                        else:
                            nc.sync.dma_start(qout[:, o0:o0 + GW],
                                              OUT[:, o0:o0 + GW])

    nc.compile()
    return nc


def _get_nc(nonce=""):
    if nonce not in _CACHE:
        _CACHE[nonce] = build_nc(nonce)
    return _CACHE[nonce]


def host_prep(emit, trans, BOS):
    """f64 host prep: constants, chunk-0 scan, per-core input maps."""
    emit = emit.astype(np.float64)
    trans = trans.astype(np.float64)
    BOS = BOS.astype(np.float64)

    c0 = float(np.log(np.exp(trans).sum(0).mean()))
    E = np.exp(trans - c0)
    wk = 0.0
    if W_DT == "fp8":
        wk = float(np.floor(np.log2(200.0 / E.max())))
    Es = E * (2.0 ** wk)

    m_f = np.log(np.exp(emit).mean(axis=1))            # [T]
    D = np.exp(emit - m_f[:, None])                    # [T, NT] (fp8 range)

    # host chunk 0: factors 1..L-1 exact, log domain
    a = BOS + emit[0]
    for f in range(1, L):
        z = trans + a[:, None]
        mm = z.max(axis=0)
        a = emit[f] + np.log(np.exp(z - mm).sum(axis=0)) + mm
    a0m = float(a.max())
    v_host = np.exp(a - a0m)

    wdt = FP8 if W_DT == "fp8" else BF16

    def wlayout(lhsT):
        # [P, 4P], col = (kh*2+jt)*128 + j'
        out = np.empty((P, 4 * P), dtype=np.float64)
        for kh in range(2):
            for jt in range(2):
                out[:, (kh * 2 + jt) * P:(kh * 2 + jt + 1) * P] = \
                    lhsT[kh * P:(kh + 1) * P, jt * P:(jt + 1) * P]
        return out.astype(wdt)

    w_fwd = wlayout(Es)      # lhsT = E
    w_bwd = wlayout(Es.T)    # lhsT = E^T

    idx = np.arange(L)
    in_maps = []
    for core in range(N_CORES):
        fwd = core < 4
        k = core % 4
        # local slot s -> chunk c = k*NCH + s + 1 (clamped; slot 1023 dummy)
        chunks = np.minimum(k * NCH + np.arange(NCH) + 1, C - 1)
        f0 = chunks * L                                 # [NCH]
        # dall[:, 0] = init vector (round-0 state), dall[:, 1+i] = round-i scale
        dall = np.ones((NCH, L + 1, NT))
        if fwd:
            dall[:, 1:] = D[(f0[:, None] + idx)]
        else:
            dall[:, 0] = D[f0 + L - 1]
            dall[:, 1:L] = D[(f0[:, None] + (L - 2 - np.arange(L - 1)))]
        # demi layout [P, g*((L+1)*GW) + blk*GW + h*GCH + ch]
        X = dall.reshape(G, GCH, L + 1, 2, P)           # [g,ch,blk,h,p]
        dem = np.ascontiguousarray(
            X.transpose(4, 0, 2, 3, 1).reshape(P, G * (L + 1) * 2 * GCH))
        in_maps.append({
            "w": np.ascontiguousarray(w_fwd if fwd else w_bwd),
            "demi": np.clip(dem, 0.0, 240.0).astype(FP8),
        })
    return in_maps, dict(c0=c0, m_f=m_f, a0m=a0m, v_host=v_host,
                         wkl=wk * np.log(2.0))


def host_combine(results, aux):
    """f64 splice of probe vectors into logZ."""
    c0, m_f, a0m, v_host = aux["c0"], aux["m_f"], aux["a0m"], aux["v_host"]
    # gather vectors: qout [P, g*GW + h*GCH + ch] -> v[c][j=h*128+p]
    vs = np.empty((2, C, NT))
    for d in range(2):
        for core in range(4):
            q = results[d * 4 + core]["qout"].astype(np.float64)
            Q = q.reshape(P, G, 2, GCH).transpose(1, 3, 2, 0)  # [g,ch,h,p]
            Qr = Q.reshape(NCH, NT)
            c_start = core * NCH + 1
            n = min(NCH, C - c_start)
            vs[d, c_start:c_start + n] = Qr[:n]
    acc = a0m
    v_prev = v_host
    # per-chunk constants (each factor contributes m_f + c0 - wk*ln2)
    mc = (m_f.reshape(C, L) + c0 - aux["wkl"]).sum(axis=1)
    for c in range(1, C):
        acc += float(np.log(vs[1, c] @ v_prev)) + float(mc[c])
        v_prev = vs[0, c] / float(vs[0, c].sum())
    acc += float(np.log(v_prev.sum()))
    return acc


def gold_score(emit, y, trans, BOS, EOS):
    e = emit.astype(np.float64)
    t = trans.astype(np.float64)
    yy = np.asarray(y).astype(np.int64)
    T = e.shape[0]
    s = float(BOS[yy[0]])
    s += t[yy[:-1], yy[1:]].sum()
    s += e[np.arange(T - 1), yy[:-1]].sum()
    s += float(EOS[yy[-1]]) + e[T - 1, yy[-1]]
    return s


def kernel(emit, y, trans, BOS, EOS):
    emit = np.asarray(emit)
    trans = np.asarray(trans)
    BOS = np.asarray(BOS)
    EOS = np.asarray(EOS)
    nc = _get_nc()
    in_maps, aux = host_prep(emit, trans, BOS)
    results = run_bass_kernel_spmd(nc, in_maps, list(range(N_CORES))).results
    logZ = host_combine(results, aux)
    gold = gold_score(emit, y, trans, BOS, EOS)
    return np.array(np.float32(logZ - gold))


def prof_setup(inputs, nonce="p1"):
    """Hook for profile_hw: fresh-NEFF nc + per-core in_maps."""
    nc = _get_nc(nonce)
    in_maps, _ = host_prep(np.asarray(inputs["emit"]),
                           np.asarray(inputs["trans"]),
                           np.asarray(inputs["BOS"]))
    if nonce:
        for m in in_maps:
            m["w" + nonce] = m.pop("w")
    return nc, in_maps


# revision 40
# speedup vs baseline: 1.0645x; 1.0645x over previous
"""CRF partition-function kernel for Trainium2 (8 NeuronCores).

Probe/rank-1 splice algorithm:
  logZ = lse(alpha_{T-1}) with alpha_t = D_t E^T alpha_{t-1},
  E = exp(trans - c0), D_t = diag(exp(emit_t - m_t)) (host-centered).
  A product of >=~10 of these positive transfer matrices is numerically
  rank-1 (Perron collapse), so each chunk product P_c (L=8 factors) is
  fully described by two probe vectors:
      v_c = P_c @ 1   (forward vector scan)
      r_c = P_c^T @ 1 (backward vector scan)
  with P_c ~= v_c r_c^T / (1^T v_c), and
      logZ = log(r_1^T alpha_host) + sum_c log(r_{c+1}^T v_c / 1^T v_c)
             + sum over device factors (m_f + c0) + host-chunk part.
  This replaces the T*NT^3 matrix scan with 2*T*NT^2 of batched
  matvecs.  T=8192 -> 2048 chunks of L=4; chunk 0 (3 factors, exact
  BOS start) runs on the host in f64; the other 2047 forward and 2047
  backward chains run as 8 cores x 512 lockstep chains x 4 rounds
  (cores 0-3 forward, 4-7 backward; identical program, different
  inputs -- the backward recurrence z <- E(d*z) is re-shaped to
  MM-then-scale by a one-column emission shift, and each chain's init
  vector rides in the demi tensor as a leading block).

Round (per chain group g of 256 chains): one PSUM tile [128, 512]
accumulates 4 matmuls (2 j-tiles x 2 k-halves), then one VectorE
tensor_mul applies the per-(state,chain) emission column and writes
the bf16 state.  A dummy-matmul warmup burst opens the PE HAM clock
gate during the input DMAs; demi streams on 3 DMA queues in
need-order; qout is dumped per group across 3 queues.

Dtypes: weights fp8e4 (rescaled so max ~200), emissions fp8e4 (the
2^-wk compensation is left out so the bf16 state grows 2^wk/round;
host constants account for it), state bf16, PSUM f32.  Validated vs
f64 reference: ~-10 nats on logZ ~53616 (tolerance 2e-2 rel ~= 1070).
"""

import numpy as np
import ml_dtypes

import concourse.bass as bass
import concourse.bacc as bacc
import concourse.mybir as mybir
import concourse.tile as tile
from concourse.bass_utils import run_bass_kernel_spmd

BF16 = ml_dtypes.bfloat16
FP8 = ml_dtypes.float8_e4m3

NT = 256
T_FULL = 8192
N_CORES = 8
P = 128
L = 4             # rounds (chunk length)
C = T_FULL // L   # 2048 chunks (chunk 0 on host)
NCH = 512         # chain slots per core
G = 2             # chain groups (latency hiding)
GCH = NCH // G
RB = 2            # demi round-blocks per group
RL = L // RB
W_DT = "fp8"      # "fp8" | "bf16"

_CACHE = {}


def build_nc(nonce=""):
    f32 = mybir.dt.float32
    bf16 = mybir.dt.bfloat16
    wdt = mybir.dt.float8e4 if W_DT == "fp8" else bf16
    GW = 2 * GCH          # state/psum width per group
    DBW = RL * GW         # demi block width

    fp8 = mybir.dt.float8e4
    nc = bacc.Bacc(None, target_bir_lowering=False)
    # w: col = (kh*2+jt)*128 + j'   (lhsT blocks)
    w = nc.declare_dram_parameter("w" + nonce, [P, 4 * P], wdt, isOutput=False)
    # demi: per group [init | round 0 | ... | round L-1] blocks of GW cols,
    # each block col = h*GCH + ch.  Init doubles as the round-0 state.
    # fp8, with the 2^-wk weight compensation left out (the bf16 state
    # simply grows 2^wk per round; host constants account for it).
    demi = nc.declare_dram_parameter(
        "demi", [P, G * (L + 1) * GW], fp8, isOutput=False)
    qout = nc.declare_dram_parameter("qout", [P, G * GW], bf16, isOutput=True)

    with tile.TileContext(nc) as tc:
        with (
            tc.tile_pool(name="const", bufs=1) as cp,
            tc.tile_pool(name="state", bufs=1) as sp,
            tc.tile_pool(name="psA", bufs=2, space=bass.MemorySpace.PSUM) as ppA,
            tc.tile_pool(name="psB", bufs=2, space=bass.MemorySpace.PSUM) as ppB,
        ):
            # PE warmup burst: ~3.4us of dummy matmuls (no data deps) so the
            # HAM clock gate opens before the real rounds start.  The memset
            # goes on GpSimd, which clears its framework preamble ~1us
            # earlier than Vector, so the burst starts sooner.
            wu = sp.tile([P, 5 * P], bf16, tag="wu", name="wu")
            nc.gpsimd.memset(wu[:], 1.0)
            wups = ppA.tile([P, 4 * P], f32, tag="wups", name="wups")
            for _ in range(5):
                nc.tensor.matmul(wups[:], wu[:, 0:P], wu[:, P:5 * P],
                                 start=True, stop=True)

            Wt = cp.tile([P, 4 * P], wdt, tag="w", name="w")
            nc.sync.dma_start(Wt[:], w[:, :])

            S = [[sp.tile([P, GW], bf16, tag=f"s{g}{ph}", name=f"s{g}{ph}")
                  for ph in range(2)] for g in range(G)]
            OUT = sp.tile([P, G * GW], bf16, tag="out", name="out")

            # demi: issued in need-order per group (init gates round 0);
            # sync takes the last block so the two group queues stay short
            GB = (L + 1) * GW
            Dinit = [cp.tile([P, GW], fp8, tag=f"di{g}", name=f"di{g}")
                     for g in range(G)]
            D = [[cp.tile([P, DBW], fp8, tag=f"d{g}{b}", name=f"d{g}{b}")
                  for b in range(RB)] for g in range(G)]
            dq = {(0, 0): nc.scalar, (0, 1): nc.scalar,
                  (1, 0): nc.gpsimd, (1, 1): nc.sync}
            for g in range(G):
                eng = nc.scalar if g == 0 else nc.sync
                eng.dma_start(Dinit[g][:], demi[:, g * GB:g * GB + GW])
            for b in range(RB):
                for g in range(G):
                    dq[(g, b)].dma_start(
                        D[g][b][:],
                        demi[:, g * GB + (1 + b * RL) * GW:
                             g * GB + (1 + (b + 1) * RL) * GW])

            pools = [ppA, ppB]
            for i in range(L):
                b, ri = divmod(i, RL)
                PS = [pools[g].tile([P, GW], f32, tag=f"ps{g}", name=f"ps{g}")
                      for g in range(G)]
                for g in range(G):
                    for kh, jt in ((0, 0), (1, 0), (0, 1), (1, 1)):
                        wsl = Wt[:, (kh * 2 + jt) * P:(kh * 2 + jt + 1) * P]
                        if i == 0:
                            rhs = Dinit[g][:, kh * GCH:(kh + 1) * GCH]
                        else:
                            rhs = S[g][(i + 1) % 2][:, kh * GCH:(kh + 1) * GCH]
                        nc.tensor.matmul(
                            PS[g][:, jt * GCH:(jt + 1) * GCH],
                            wsl, rhs,
                            start=(kh == 0), stop=(kh == 1))
                for g in range(G):
                    dst = (OUT[:, g * GW:(g + 1) * GW] if i == L - 1
                           else S[g][i % 2][:])
                    nc.vector.tensor_mul(
                        dst, PS[g][:],
                        D[g][b][:, ri * GW:(ri + 1) * GW])
                    if i == L - 1:
                        # dump this group's half as soon as its TT lands;
                        # g0 split across two queues for parallel rings
                        # (gpsimd's slow drain overlaps the teardown)
                        o0 = g * GW
                        if g == 0:
                            h = GW // 2
                            nc.scalar.dma_start(qout[:, o0:o0 + h],
                                                OUT[:, o0:o0 + h])
                            nc.gpsimd.dma_start(qout[:, o0 + h:o0 + GW],
                                                OUT[:, o0 + h:o0 + GW])
                        else:
                            nc.sync.dma_start(qout[:, o0:o0 + GW],
                                              OUT[:, o0:o0 + GW])

    nc.compile()
    return nc


def _get_nc(nonce=""):
    if nonce not in _CACHE:
        _CACHE[nonce] = build_nc(nonce)
    return _CACHE[nonce]


def host_prep(emit, trans, BOS):
    """f64 host prep: constants, chunk-0 scan, per-core input maps."""
    emit = emit.astype(np.float64)
    trans = trans.astype(np.float64)
    BOS = BOS.astype(np.float64)

    c0 = float(np.log(np.exp(trans).sum(0).mean()))
    E = np.exp(trans - c0)
    wk = 0.0
    if W_DT == "fp8":
        wk = float(np.floor(np.log2(200.0 / E.max())))
    Es = E * (2.0 ** wk)

    m_f = np.log(np.exp(emit).mean(axis=1))            # [T]
    D = np.exp(emit - m_f[:, None])                    # [T, NT] (fp8 range)

    # host chunk 0: factors 1..L-1 exact, log domain
    a = BOS + emit[0]
    for f in range(1, L):
        z = trans + a[:, None]
        mm = z.max(axis=0)
        a = emit[f] + np.log(np.exp(z - mm).sum(axis=0)) + mm
    a0m = float(a.max())
    v_host = np.exp(a - a0m)

    wdt = FP8 if W_DT == "fp8" else BF16

    def wlayout(lhsT):
        # [P, 4P], col = (kh*2+jt)*128 + j'
        out = np.empty((P, 4 * P), dtype=np.float64)
        for kh in range(2):
            for jt in range(2):
                out[:, (kh * 2 + jt) * P:(kh * 2 + jt + 1) * P] = \
                    lhsT[kh * P:(kh + 1) * P, jt * P:(jt + 1) * P]
        return out.astype(wdt)

    w_fwd = wlayout(Es)      # lhsT = E
    w_bwd = wlayout(Es.T)    # lhsT = E^T

    idx = np.arange(L)
    in_maps = []
    for core in range(N_CORES):
        fwd = core < 4
        k = core % 4
        # local slot s -> chunk c = k*NCH + s + 1 (clamped; slot 1023 dummy)
        chunks = np.minimum(k * NCH + np.arange(NCH) + 1, C - 1)
        f0 = chunks * L                                 # [NCH]
        # dall[:, 0] = init vector (round-0 state), dall[:, 1+i] = round-i scale
        dall = np.ones((NCH, L + 1, NT))
        if fwd:
            dall[:, 1:] = D[(f0[:, None] + idx)]
        else:
            dall[:, 0] = D[f0 + L - 1]
            dall[:, 1:L] = D[(f0[:, None] + (L - 2 - np.arange(L - 1)))]
        # demi layout [P, g*((L+1)*GW) + blk*GW + h*GCH + ch]
        X = dall.reshape(G, GCH, L + 1, 2, P)           # [g,ch,blk,h,p]
        dem = np.ascontiguousarray(
            X.transpose(4, 0, 2, 3, 1).reshape(P, G * (L + 1) * 2 * GCH))
        in_maps.append({
            "w": np.ascontiguousarray(w_fwd if fwd else w_bwd),
            "demi": np.clip(dem, 0.0, 240.0).astype(FP8),
        })
    return in_maps, dict(c0=c0, m_f=m_f, a0m=a0m, v_host=v_host,
                         wkl=wk * np.log(2.0))


def host_combine(results, aux):
    """f64 splice of probe vectors into logZ."""
    c0, m_f, a0m, v_host = aux["c0"], aux["m_f"], aux["a0m"], aux["v_host"]
    # gather vectors: qout [P, g*GW + h*GCH + ch] -> v[c][j=h*128+p]
    vs = np.empty((2, C, NT))
    for d in range(2):
        for core in range(4):
            q = results[d * 4 + core]["qout"].astype(np.float64)
            Q = q.reshape(P, G, 2, GCH).transpose(1, 3, 2, 0)  # [g,ch,h,p]
            Qr = Q.reshape(NCH, NT)
            c_start = core * NCH + 1
            n = min(NCH, C - c_start)
            vs[d, c_start:c_start + n] = Qr[:n]
    acc = a0m
    v_prev = v_host
    # per-chunk constants (each factor contributes m_f + c0 - wk*ln2)
    mc = (m_f.reshape(C, L) + c0 - aux["wkl"]).sum(axis=1)
    for c in range(1, C):
        acc += float(np.log(vs[1, c] @ v_prev)) + float(mc[c])
        v_prev = vs[0, c] / float(vs[0, c].sum())
    acc += float(np.log(v_prev.sum()))
    return acc


def gold_score(emit, y, trans, BOS, EOS):
    e = emit.astype(np.float64)
    t = trans.astype(np.float64)
    yy = np.asarray(y).astype(np.int64)
    T = e.shape[0]
    s = float(BOS[yy[0]])
    s += t[yy[:-1], yy[1:]].sum()
    s += e[np.arange(T - 1), yy[:-1]].sum()
    s += float(EOS[yy[-1]]) + e[T - 1, yy[-1]]
    return s


def kernel(emit, y, trans, BOS, EOS):
    emit = np.asarray(emit)
    trans = np.asarray(trans)
    BOS = np.asarray(BOS)
    EOS = np.asarray(EOS)
    nc = _get_nc()
    in_maps, aux = host_prep(emit, trans, BOS)
    results = run_bass_kernel_spmd(nc, in_maps, list(range(N_CORES))).results
    logZ = host_combine(results, aux)
    gold = gold_score(emit, y, trans, BOS, EOS)
    return np.array(np.float32(logZ - gold))


def prof_setup(inputs, nonce="p1"):
    """Hook for profile_hw: fresh-NEFF nc + per-core in_maps."""
    nc = _get_nc(nonce)
    in_maps, _ = host_prep(np.asarray(inputs["emit"]),
                           np.asarray(inputs["trans"]),
                           np.asarray(inputs["BOS"]))
    if nonce:
        for m in in_maps:
            m["w" + nonce] = m.pop("w")
    return nc, in_maps


# revision 44
# speedup vs baseline: 1.1154x; 1.0478x over previous
"""CRF partition-function kernel for Trainium2 (8 NeuronCores).

Probe/rank-1 splice algorithm:
  logZ = lse(alpha_{T-1}) with alpha_t = D_t E^T alpha_{t-1},
  E = exp(trans - c0), D_t = diag(exp(emit_t - m_t)) (host-centered).
  A product of >=~10 of these positive transfer matrices is numerically
  rank-1 (Perron collapse), so each chunk product P_c (L=8 factors) is
  fully described by two probe vectors:
      v_c = P_c @ 1   (forward vector scan)
      r_c = P_c^T @ 1 (backward vector scan)
  with P_c ~= v_c r_c^T / (1^T v_c), and
      logZ = log(r_1^T alpha_host) + sum_c log(r_{c+1}^T v_c / 1^T v_c)
             + sum over device factors (m_f + c0) + host-chunk part.
  This replaces the T*NT^3 matrix scan with 2*T*NT^2 of batched
  matvecs.  T=8192 -> 2048 chunks of L=4; chunk 0 (3 factors, exact
  BOS start) runs on the host in f64; the other 2047 forward and 2047
  backward chains run as 8 cores x 512 lockstep chains x 4 rounds
  (cores 0-3 forward, 4-7 backward; identical program, different
  inputs -- the backward recurrence z <- E(d*z) is re-shaped to
  MM-then-scale by a one-column emission shift, and each chain's init
  vector rides in the demi tensor as a leading block).

Round (per chain group g of 256 chains): one PSUM tile [128, 512]
accumulates 4 matmuls (2 j-tiles x 2 k-halves), then one VectorE
tensor_mul applies the per-(state,chain) emission column and writes
the bf16 state.  A dummy-matmul warmup burst opens the PE HAM clock
gate during the input DMAs; demi streams on 3 DMA queues in
need-order; qout is dumped per group across 3 queues.

Dtypes: weights fp8e4 (rescaled so max ~200), emissions fp8e4 (the
2^-wk compensation is left out so the bf16 state grows 2^wk/round;
host constants account for it), state bf16, PSUM f32.  Validated vs
f64 reference: ~-10 nats on logZ ~53616 (tolerance 2e-2 rel ~= 1070).
"""

import numpy as np
import ml_dtypes

import concourse.bass as bass
import concourse.bacc as bacc
import concourse.mybir as mybir
import concourse.tile as tile
from concourse.bass_utils import run_bass_kernel_spmd

BF16 = ml_dtypes.bfloat16
FP8 = ml_dtypes.float8_e4m3

NT = 256
T_FULL = 8192
N_CORES = 8
P = 128
L = 4             # rounds (chunk length)
C = T_FULL // L   # 2048 chunks (chunk 0 on host)
NCH = 512         # chain slots per core
G = 2             # chain groups (latency hiding)
GCH = NCH // G
RB = 2            # demi round-blocks per group
RL = L // RB
W_DT = "fp8"      # "fp8" | "bf16"

_CACHE = {}


def build_nc(nonce=""):
    f32 = mybir.dt.float32
    bf16 = mybir.dt.bfloat16
    wdt = mybir.dt.float8e4 if W_DT == "fp8" else bf16
    GW = 2 * GCH          # state/psum width per group
    DBW = RL * GW         # demi block width

    fp8 = mybir.dt.float8e4
    nc = bacc.Bacc(None, target_bir_lowering=False)
    # w: col = (kh*2+jt)*128 + j'   (lhsT blocks)
    w = nc.declare_dram_parameter("w" + nonce, [P, 4 * P], wdt, isOutput=False)
    # demi: per group [init | round 0 | ... | round L-1] blocks of GW cols,
    # each block col = h*GCH + ch.  Init doubles as the round-0 state.
    # fp8, with the 2^-wk weight compensation left out (the bf16 state
    # simply grows 2^wk per round; host constants account for it).
    demi = nc.declare_dram_parameter(
        "demi", [P, G * (L + 1) * GW], fp8, isOutput=False)
    qout = nc.declare_dram_parameter("qout", [P, G * GW], bf16, isOutput=True)

    with tile.TileContext(nc) as tc:
        with (
            tc.tile_pool(name="const", bufs=1) as cp,
            tc.tile_pool(name="state", bufs=1) as sp,
            tc.tile_pool(name="psA", bufs=2, space=bass.MemorySpace.PSUM) as ppA,
            tc.tile_pool(name="psB", bufs=2, space=bass.MemorySpace.PSUM) as ppB,
        ):
            # PE warmup burst: ~2.6us of dummy matmuls (no data deps) that
            # fill the input-DMA dead time and start opening the HAM clock
            # gate before the real rounds begin.
            wu = sp.tile([P, 5 * P], bf16, tag="wu", name="wu")
            nc.vector.memset(wu[:], 1.0)
            wups = ppA.tile([P, 4 * P], f32, tag="wups", name="wups")
            for _ in range(6):
                nc.tensor.matmul(wups[:], wu[:, 0:P], wu[:, P:5 * P],
                                 start=True, stop=True)

            Wt = cp.tile([P, 4 * P], wdt, tag="w", name="w")
            nc.sync.dma_start(Wt[:], w[:, :])

            S = [[sp.tile([P, GW], bf16, tag=f"s{g}{ph}", name=f"s{g}{ph}")
                  for ph in range(2)] for g in range(G)]
            OUT = sp.tile([P, G * GW], bf16, tag="out", name="out")

            # demi: issued in need-order per group (init gates round 0);
            # sync takes the last block so the two group queues stay short
            GB = (L + 1) * GW
            Dinit = [cp.tile([P, GW], fp8, tag=f"di{g}", name=f"di{g}")
                     for g in range(G)]
            D = [[cp.tile([P, DBW], fp8, tag=f"d{g}{b}", name=f"d{g}{b}")
                  for b in range(RB)] for g in range(G)]
            dq = {(0, 0): nc.scalar, (0, 1): nc.scalar,
                  (1, 0): nc.gpsimd, (1, 1): nc.sync}
            for g in range(G):
                eng = nc.scalar if g == 0 else nc.sync
                eng.dma_start(Dinit[g][:], demi[:, g * GB:g * GB + GW])
            for b in range(RB):
                for g in range(G):
                    dq[(g, b)].dma_start(
                        D[g][b][:],
                        demi[:, g * GB + (1 + b * RL) * GW:
                             g * GB + (1 + (b + 1) * RL) * GW])

            pools = [ppA, ppB]
            for i in range(L):
                b, ri = divmod(i, RL)
                PS = [pools[g].tile([P, GW], f32, tag=f"ps{g}", name=f"ps{g}")
                      for g in range(G)]
                for g in range(G):
                    for kh, jt in ((0, 0), (1, 0), (0, 1), (1, 1)):
                        wsl = Wt[:, (kh * 2 + jt) * P:(kh * 2 + jt + 1) * P]
                        if i == 0:
                            rhs = Dinit[g][:, kh * GCH:(kh + 1) * GCH]
                        else:
                            rhs = S[g][(i + 1) % 2][:, kh * GCH:(kh + 1) * GCH]
                        nc.tensor.matmul(
                            PS[g][:, jt * GCH:(jt + 1) * GCH],
                            wsl, rhs,
                            start=(kh == 0), stop=(kh == 1))
                for g in range(G):
                    dst = (OUT[:, g * GW:(g + 1) * GW] if i == L - 1
                           else S[g][i % 2][:])
                    nc.vector.tensor_mul(
                        dst, PS[g][:],
                        D[g][b][:, ri * GW:(ri + 1) * GW])
                    if i == L - 1:
                        # dump this group's half as soon as its TT lands;
                        # g0 split across two queues for parallel rings
                        # (gpsimd's slow drain overlaps the teardown)
                        o0 = g * GW
                        if g == 0:
                            h = GW // 2
                            nc.scalar.dma_start(qout[:, o0:o0 + h],
                                                OUT[:, o0:o0 + h])
                            nc.gpsimd.dma_start(qout[:, o0 + h:o0 + GW],
                                                OUT[:, o0 + h:o0 + GW])
                        else:
                            nc.sync.dma_start(qout[:, o0:o0 + GW],
                                              OUT[:, o0:o0 + GW])

    nc.compile()
    return nc


def _get_nc(nonce=""):
    if nonce not in _CACHE:
        _CACHE[nonce] = build_nc(nonce)
    return _CACHE[nonce]


def host_prep(emit, trans, BOS):
    """f64 host prep: constants, chunk-0 scan, per-core input maps."""
    emit = emit.astype(np.float64)
    trans = trans.astype(np.float64)
    BOS = BOS.astype(np.float64)

    c0 = float(np.log(np.exp(trans).sum(0).mean()))
    E = np.exp(trans - c0)
    wk = 0.0
    if W_DT == "fp8":
        wk = float(np.floor(np.log2(200.0 / E.max())))
    Es = E * (2.0 ** wk)

    m_f = np.log(np.exp(emit).mean(axis=1))            # [T]
    D = np.exp(emit - m_f[:, None])                    # [T, NT] (fp8 range)

    # host chunk 0: factors 1..L-1 exact, log domain
    a = BOS + emit[0]
    for f in range(1, L):
        z = trans + a[:, None]
        mm = z.max(axis=0)
        a = emit[f] + np.log(np.exp(z - mm).sum(axis=0)) + mm
    a0m = float(a.max())
    v_host = np.exp(a - a0m)

    wdt = FP8 if W_DT == "fp8" else BF16

    def wlayout(lhsT):
        # [P, 4P], col = (kh*2+jt)*128 + j'
        out = np.empty((P, 4 * P), dtype=np.float64)
        for kh in range(2):
            for jt in range(2):
                out[:, (kh * 2 + jt) * P:(kh * 2 + jt + 1) * P] = \
                    lhsT[kh * P:(kh + 1) * P, jt * P:(jt + 1) * P]
        return out.astype(wdt)

    w_fwd = wlayout(Es)      # lhsT = E
    w_bwd = wlayout(Es.T)    # lhsT = E^T

    idx = np.arange(L)
    in_maps = []
    for core in range(N_CORES):
        fwd = core < 4
        k = core % 4
        # local slot s -> chunk c = k*NCH + s + 1 (clamped; slot 1023 dummy)
        chunks = np.minimum(k * NCH + np.arange(NCH) + 1, C - 1)
        f0 = chunks * L                                 # [NCH]
        # dall[:, 0] = init vector (round-0 state), dall[:, 1+i] = round-i scale
        dall = np.ones((NCH, L + 1, NT))
        if fwd:
            dall[:, 1:] = D[(f0[:, None] + idx)]
        else:
            dall[:, 0] = D[f0 + L - 1]
            dall[:, 1:L] = D[(f0[:, None] + (L - 2 - np.arange(L - 1)))]
        # demi layout [P, g*((L+1)*GW) + blk*GW + h*GCH + ch]
        X = dall.reshape(G, GCH, L + 1, 2, P)           # [g,ch,blk,h,p]
        dem = np.ascontiguousarray(
            X.transpose(4, 0, 2, 3, 1).reshape(P, G * (L + 1) * 2 * GCH))
        in_maps.append({
            "w": np.ascontiguousarray(w_fwd if fwd else w_bwd),
            "demi": np.clip(dem, 0.0, 240.0).astype(FP8),
        })
    return in_maps, dict(c0=c0, m_f=m_f, a0m=a0m, v_host=v_host,
                         wkl=wk * np.log(2.0))


def host_combine(results, aux):
    """f64 splice of probe vectors into logZ."""
    c0, m_f, a0m, v_host = aux["c0"], aux["m_f"], aux["a0m"], aux["v_host"]
    # gather vectors: qout [P, g*GW + h*GCH + ch] -> v[c][j=h*128+p]
    vs = np.empty((2, C, NT))
    for d in range(2):
        for core in range(4):
            q = results[d * 4 + core]["qout"].astype(np.float64)
            Q = q.reshape(P, G, 2, GCH).transpose(1, 3, 2, 0)  # [g,ch,h,p]
            Qr = Q.reshape(NCH, NT)
            c_start = core * NCH + 1
            n = min(NCH, C - c_start)
            vs[d, c_start:c_start + n] = Qr[:n]
    acc = a0m
    v_prev = v_host
    # per-chunk constants (each factor contributes m_f + c0 - wk*ln2)
    mc = (m_f.reshape(C, L) + c0 - aux["wkl"]).sum(axis=1)
    for c in range(1, C):
        acc += float(np.log(vs[1, c] @ v_prev)) + float(mc[c])
        v_prev = vs[0, c] / float(vs[0, c].sum())
    acc += float(np.log(v_prev.sum()))
    return acc


def gold_score(emit, y, trans, BOS, EOS):
    e = emit.astype(np.float64)
    t = trans.astype(np.float64)
    yy = np.asarray(y).astype(np.int64)
    T = e.shape[0]
    s = float(BOS[yy[0]])
    s += t[yy[:-1], yy[1:]].sum()
    s += e[np.arange(T - 1), yy[:-1]].sum()
    s += float(EOS[yy[-1]]) + e[T - 1, yy[-1]]
    return s


def kernel(emit, y, trans, BOS, EOS):
    emit = np.asarray(emit)
    trans = np.asarray(trans)
    BOS = np.asarray(BOS)
    EOS = np.asarray(EOS)
    nc = _get_nc()
    in_maps, aux = host_prep(emit, trans, BOS)
    results = run_bass_kernel_spmd(nc, in_maps, list(range(N_CORES))).results
    logZ = host_combine(results, aux)
    gold = gold_score(emit, y, trans, BOS, EOS)
    return np.array(np.float32(logZ - gold))


def prof_setup(inputs, nonce="p1"):
    """Hook for profile_hw: fresh-NEFF nc + per-core in_maps."""
    nc = _get_nc(nonce)
    in_maps, _ = host_prep(np.asarray(inputs["emit"]),
                           np.asarray(inputs["trans"]),
                           np.asarray(inputs["BOS"]))
    if nonce:
        for m in in_maps:
            m["w" + nonce] = m.pop("w")
    return nc, in_maps


# revision 45
# speedup vs baseline: 1.1203x; 1.0044x over previous
"""CRF partition-function kernel for Trainium2 (8 NeuronCores).

Probe/rank-1 splice algorithm:
  logZ = lse(alpha_{T-1}) with alpha_t = D_t E^T alpha_{t-1},
  E = exp(trans - c0), D_t = diag(exp(emit_t - m_t)) (host-centered).
  A product of >=~10 of these positive transfer matrices is numerically
  rank-1 (Perron collapse), so each chunk product P_c (L=8 factors) is
  fully described by two probe vectors:
      v_c = P_c @ 1   (forward vector scan)
      r_c = P_c^T @ 1 (backward vector scan)
  with P_c ~= v_c r_c^T / (1^T v_c), and
      logZ = log(r_1^T alpha_host) + sum_c log(r_{c+1}^T v_c / 1^T v_c)
             + sum over device factors (m_f + c0) + host-chunk part.
  This replaces the T*NT^3 matrix scan with 2*T*NT^2 of batched
  matvecs.  T=8192 -> 2048 chunks of L=4; chunk 0 (3 factors, exact
  BOS start) runs on the host in f64; the other 2047 forward and 2047
  backward chains run as 8 cores x 512 lockstep chains x 4 rounds
  (cores 0-3 forward, 4-7 backward; identical program, different
  inputs -- the backward recurrence z <- E(d*z) is re-shaped to
  MM-then-scale by a one-column emission shift, and each chain's init
  vector rides in the demi tensor as a leading block).

Round (per chain group g of 256 chains): one PSUM tile [128, 512]
accumulates 4 matmuls (2 j-tiles x 2 k-halves), then one VectorE
tensor_mul applies the per-(state,chain) emission column and writes
the bf16 state.  A dummy-matmul warmup burst opens the PE HAM clock
gate during the input DMAs; demi streams on 3 DMA queues in
need-order; qout is dumped per group across 3 queues.

Dtypes: weights fp8e4 (rescaled so max ~200), emissions fp8e4 (the
2^-wk compensation is left out so the bf16 state grows 2^wk/round;
host constants account for it), state bf16, PSUM f32.  Validated vs
f64 reference: ~-10 nats on logZ ~53616 (tolerance 2e-2 rel ~= 1070).
"""

import numpy as np
import ml_dtypes

import concourse.bass as bass
import concourse.bacc as bacc
import concourse.mybir as mybir
import concourse.tile as tile
from concourse.bass_utils import run_bass_kernel_spmd

BF16 = ml_dtypes.bfloat16
FP8 = ml_dtypes.float8_e4m3

NT = 256
T_FULL = 8192
N_CORES = 8
P = 128
L = 4             # rounds (chunk length)
C = T_FULL // L   # 2048 chunks (chunk 0 on host)
NCH = 512         # chain slots per core
G = 2             # chain groups (latency hiding)
GCH = NCH // G
RB = 2            # demi round-blocks per group
RL = L // RB
W_DT = "fp8"      # "fp8" | "bf16"

_CACHE = {}


def build_nc(nonce=""):
    f32 = mybir.dt.float32
    bf16 = mybir.dt.bfloat16
    wdt = mybir.dt.float8e4 if W_DT == "fp8" else bf16
    GW = 2 * GCH          # state/psum width per group
    DBW = RL * GW         # demi block width

    fp8 = mybir.dt.float8e4
    nc = bacc.Bacc(None, target_bir_lowering=False)
    # w: col = (kh*2+jt)*128 + j'   (lhsT blocks)
    w = nc.declare_dram_parameter("w" + nonce, [P, 4 * P], wdt, isOutput=False)
    # demi: per group [init | round 0 | ... | round L-1] blocks of GW cols,
    # each block col = h*GCH + ch.  Init doubles as the round-0 state.
    # fp8, with the 2^-wk weight compensation left out (the bf16 state
    # simply grows 2^wk per round; host constants account for it).
    demi = nc.declare_dram_parameter(
        "demi", [P, G * (L + 1) * GW], fp8, isOutput=False)
    qout = nc.declare_dram_parameter("qout", [P, G * GW], bf16, isOutput=True)

    with tile.TileContext(nc) as tc:
        with (
            tc.tile_pool(name="const", bufs=1) as cp,
            tc.tile_pool(name="state", bufs=1) as sp,
            tc.tile_pool(name="psA", bufs=2, space=bass.MemorySpace.PSUM) as ppA,
            tc.tile_pool(name="psB", bufs=2, space=bass.MemorySpace.PSUM) as ppB,
        ):
            # PE warmup burst: ~2.6us of dummy matmuls (no data deps) that
            # fill the input-DMA dead time and start opening the HAM clock
            # gate before the real rounds begin.
            wu = sp.tile([P, 2 * P], bf16, tag="wu", name="wu")
            nc.vector.memset(wu[:], 1.0)
            wups = ppA.tile([P, 2 * P], f32, tag="wups", name="wups")
            for _ in range(12):
                nc.tensor.matmul(wups[:], wu[:, 0:P], wu[:, 0:2 * P],
                                 start=True, stop=True)

            Wt = cp.tile([P, 4 * P], wdt, tag="w", name="w")
            nc.sync.dma_start(Wt[:], w[:, :])

            S = [[sp.tile([P, GW], bf16, tag=f"s{g}{ph}", name=f"s{g}{ph}")
                  for ph in range(2)] for g in range(G)]
            OUT = sp.tile([P, G * GW], bf16, tag="out", name="out")

            # demi: issued in need-order per group (init gates round 0);
            # sync takes the last block so the two group queues stay short
            GB = (L + 1) * GW
            Dinit = [cp.tile([P, GW], fp8, tag=f"di{g}", name=f"di{g}")
                     for g in range(G)]
            D = [[cp.tile([P, DBW], fp8, tag=f"d{g}{b}", name=f"d{g}{b}")
                  for b in range(RB)] for g in range(G)]
            dq = {(0, 0): nc.scalar, (0, 1): nc.scalar,
                  (1, 0): nc.gpsimd, (1, 1): nc.sync}
            for g in range(G):
                eng = nc.scalar if g == 0 else nc.sync
                eng.dma_start(Dinit[g][:], demi[:, g * GB:g * GB + GW])
            for b in range(RB):
                for g in range(G):
                    dq[(g, b)].dma_start(
                        D[g][b][:],
                        demi[:, g * GB + (1 + b * RL) * GW:
                             g * GB + (1 + (b + 1) * RL) * GW])

            pools = [ppA, ppB]
            for i in range(L):
                b, ri = divmod(i, RL)
                PS = [pools[g].tile([P, GW], f32, tag=f"ps{g}", name=f"ps{g}")
                      for g in range(G)]
                for g in range(G):
                    for kh, jt in ((0, 0), (1, 0), (0, 1), (1, 1)):
                        wsl = Wt[:, (kh * 2 + jt) * P:(kh * 2 + jt + 1) * P]
                        if i == 0:
                            rhs = Dinit[g][:, kh * GCH:(kh + 1) * GCH]
                        else:
                            rhs = S[g][(i + 1) % 2][:, kh * GCH:(kh + 1) * GCH]
                        nc.tensor.matmul(
                            PS[g][:, jt * GCH:(jt + 1) * GCH],
                            wsl, rhs,
                            start=(kh == 0), stop=(kh == 1))
                for g in range(G):
                    dst = (OUT[:, g * GW:(g + 1) * GW] if i == L - 1
                           else S[g][i % 2][:])
                    nc.vector.tensor_mul(
                        dst, PS[g][:],
                        D[g][b][:, ri * GW:(ri + 1) * GW])
                    if i == L - 1:
                        # dump this group's half as soon as its TT lands;
                        # g0 split across two queues for parallel rings
                        # (gpsimd's slow drain overlaps the teardown)
                        o0 = g * GW
                        if g == 0:
                            h = GW // 2
                            nc.scalar.dma_start(qout[:, o0:o0 + h],
                                                OUT[:, o0:o0 + h])
                            nc.gpsimd.dma_start(qout[:, o0 + h:o0 + GW],
                                                OUT[:, o0 + h:o0 + GW])
                        else:
                            nc.sync.dma_start(qout[:, o0:o0 + GW],
                                              OUT[:, o0:o0 + GW])

    nc.compile()
    return nc


def _get_nc(nonce=""):
    if nonce not in _CACHE:
        _CACHE[nonce] = build_nc(nonce)
    return _CACHE[nonce]


def host_prep(emit, trans, BOS):
    """f64 host prep: constants, chunk-0 scan, per-core input maps."""
    emit = emit.astype(np.float64)
    trans = trans.astype(np.float64)
    BOS = BOS.astype(np.float64)

    c0 = float(np.log(np.exp(trans).sum(0).mean()))
    E = np.exp(trans - c0)
    wk = 0.0
    if W_DT == "fp8":
        wk = float(np.floor(np.log2(200.0 / E.max())))
    Es = E * (2.0 ** wk)

    m_f = np.log(np.exp(emit).mean(axis=1))            # [T]
    D = np.exp(emit - m_f[:, None])                    # [T, NT] (fp8 range)

    # host chunk 0: factors 1..L-1 exact, log domain
    a = BOS + emit[0]
    for f in range(1, L):
        z = trans + a[:, None]
        mm = z.max(axis=0)
        a = emit[f] + np.log(np.exp(z - mm).sum(axis=0)) + mm
    a0m = float(a.max())
    v_host = np.exp(a - a0m)

    wdt = FP8 if W_DT == "fp8" else BF16

    def wlayout(lhsT):
        # [P, 4P], col = (kh*2+jt)*128 + j'
        out = np.empty((P, 4 * P), dtype=np.float64)
        for kh in range(2):
            for jt in range(2):
                out[:, (kh * 2 + jt) * P:(kh * 2 + jt + 1) * P] = \
                    lhsT[kh * P:(kh + 1) * P, jt * P:(jt + 1) * P]
        return out.astype(wdt)

    w_fwd = wlayout(Es)      # lhsT = E
    w_bwd = wlayout(Es.T)    # lhsT = E^T

    idx = np.arange(L)
    in_maps = []
    for core in range(N_CORES):
        fwd = core < 4
        k = core % 4
        # local slot s -> chunk c = k*NCH + s + 1 (clamped; slot 1023 dummy)
        chunks = np.minimum(k * NCH + np.arange(NCH) + 1, C - 1)
        f0 = chunks * L                                 # [NCH]
        # dall[:, 0] = init vector (round-0 state), dall[:, 1+i] = round-i scale
        dall = np.ones((NCH, L + 1, NT))
        if fwd:
            dall[:, 1:] = D[(f0[:, None] + idx)]
        else:
            dall[:, 0] = D[f0 + L - 1]
            dall[:, 1:L] = D[(f0[:, None] + (L - 2 - np.arange(L - 1)))]
        # demi layout [P, g*((L+1)*GW) + blk*GW + h*GCH + ch]
        X = dall.reshape(G, GCH, L + 1, 2, P)           # [g,ch,blk,h,p]
        dem = np.ascontiguousarray(
            X.transpose(4, 0, 2, 3, 1).reshape(P, G * (L + 1) * 2 * GCH))
        in_maps.append({
            "w": np.ascontiguousarray(w_fwd if fwd else w_bwd),
            "demi": np.clip(dem, 0.0, 240.0).astype(FP8),
        })
    return in_maps, dict(c0=c0, m_f=m_f, a0m=a0m, v_host=v_host,
                         wkl=wk * np.log(2.0))


def host_combine(results, aux):
    """f64 splice of probe vectors into logZ."""
    c0, m_f, a0m, v_host = aux["c0"], aux["m_f"], aux["a0m"], aux["v_host"]
    # gather vectors: qout [P, g*GW + h*GCH + ch] -> v[c][j=h*128+p]
    vs = np.empty((2, C, NT))
    for d in range(2):
        for core in range(4):
            q = results[d * 4 + core]["qout"].astype(np.float64)
            Q = q.reshape(P, G, 2, GCH).transpose(1, 3, 2, 0)  # [g,ch,h,p]
            Qr = Q.reshape(NCH, NT)
            c_start = core * NCH + 1
            n = min(NCH, C - c_start)
            vs[d, c_start:c_start + n] = Qr[:n]
    acc = a0m
    v_prev = v_host
    # per-chunk constants (each factor contributes m_f + c0 - wk*ln2)
    mc = (m_f.reshape(C, L) + c0 - aux["wkl"]).sum(axis=1)
    for c in range(1, C):
        acc += float(np.log(vs[1, c] @ v_prev)) + float(mc[c])
        v_prev = vs[0, c] / float(vs[0, c].sum())
    acc += float(np.log(v_prev.sum()))
    return acc


def gold_score(emit, y, trans, BOS, EOS):
    e = emit.astype(np.float64)
    t = trans.astype(np.float64)
    yy = np.asarray(y).astype(np.int64)
    T = e.shape[0]
    s = float(BOS[yy[0]])
    s += t[yy[:-1], yy[1:]].sum()
    s += e[np.arange(T - 1), yy[:-1]].sum()
    s += float(EOS[yy[-1]]) + e[T - 1, yy[-1]]
    return s


def kernel(emit, y, trans, BOS, EOS):
    emit = np.asarray(emit)
    trans = np.asarray(trans)
    BOS = np.asarray(BOS)
    EOS = np.asarray(EOS)
    nc = _get_nc()
    in_maps, aux = host_prep(emit, trans, BOS)
    results = run_bass_kernel_spmd(nc, in_maps, list(range(N_CORES))).results
    logZ = host_combine(results, aux)
    gold = gold_score(emit, y, trans, BOS, EOS)
    return np.array(np.float32(logZ - gold))


def prof_setup(inputs, nonce="p1"):
    """Hook for profile_hw: fresh-NEFF nc + per-core in_maps."""
    nc = _get_nc(nonce)
    in_maps, _ = host_prep(np.asarray(inputs["emit"]),
                           np.asarray(inputs["trans"]),
                           np.asarray(inputs["BOS"]))
    if nonce:
        for m in in_maps:
            m["w" + nonce] = m.pop("w")
    return nc, in_maps
